# revision 1
# baseline (speedup 1.0000x reference)
"""Dilated LSTM (B=8, T=2048, C=1024, H=1024, D=4) on 8 trn2 NeuronCores.

Strategy: data-parallel over batch (core c <- batch item c, its 4 dilation
chains). Everything is core-local.

Per core, one fused pipeline:
  Input GEMM (phase A, transposed): xgT[g, t] = W_ih @ x^T + (b_ih + b_hh)
    as 128 psum tiles [128 gate-rows, 512 t-cols]: stationary W_ih tiles
    (bf16, fully resident), moving x^T tiles (bf16, 1 cycle/row). The
    psum->SBUF evacuation runs on ACT with the bias fused as a per-partition
    scalar; tiles are staged to DRAM as xgT[32, 128, T] (fp16).
    The first 256 t-cols (steps 0..63) are computed up front (~45us head);
    the remaining tiles' matmuls/evacs/stores are interleaved a few per
    recurrence step (deadline-scheduled per t-range), filling engine idle
    windows, so the rest of phase A costs almost no wall-clock.
  Recurrence (phase B, transposed): per step l the gate pre-activations are
    computed TRANSPOSED, G^T[4096, 4] as 32 psum chunks of [128, 4]:
      G^T[c] = sum_k W_hh^T[k-tile, c-chunk] (stationary, fp16)
                     @ hT[k-tile][128, 4] (moving, fp16)        (8 matmuls)
             + I128 @ xgT[bank-chunks, 4l:4l+4] (identity fold seeds each
                     gate bank's open psum group with the x-gates, issued
                     BEFORE h(l-1) lands so it is off the critical path)
    Cost-model cost per matmul is out-free-size rows (4), so a step is
    ~288 tiny matmuls instead of streaming all of W_hh (32768 rows).
    ACT sigmoid/tanh and DVE c/h updates then run on [128, 32..96]-shaped
    tiles (gates live across partitions). h is produced directly in the
    hT [128, 4-per-k-tile] layout the next step's matmuls consume - no
    per-step transpose at all. h (fp16) accumulates in a 32-step SBUF block
    that is DMA'd to a transposed DRAM layout yT[128, 8, T]; the host
    un-transposes.
"""

import os
import sys

sys.path.insert(0, "/opt/trn_rl_repo")

import numpy as np

B, T, C, H, D = 8, 2048, 1024, 1024, 4
L = T // D  # 512 steps
G4 = 4 * H  # 4096 gates
KT = C // 128  # 8 k-tiles for x / h
NCORES = 8
PF = 4  # xg prefetch depth (steps)
BLK = 32  # y writeback block (steps)
NTT = 4  # phase A t-tiles of 512 per gate chunk
APS = 4  # interleaved phase-A matmuls per step

_cached = {}

N_STEPS = int(os.environ.get("DLSTM_STEPS", str(L)))  # dev override only


def _pcol(c):
    """col group within the step's psum banks. if bank: i(0..7)->0..7,
    f(8..15)->8..15; g~ bank: g~(16..23)->0..7; o bank: o(24..31)->0..7.
    (psum accumulation groups are tracked per 2KB bank; each ACT read
    must target a bank whose groups are all closed, so the three ACT
    reads per step get three separate banks)"""
    if c < 16:
        return c
    if c >= 24:
        return c - 24
    return c - 16


def _build(n_steps):
    import concourse.bass as bass
    import concourse.bacc as bacc
    import concourse.mybir as mybir
    from contextlib import ExitStack

    F32 = mybir.dt.float32
    F16 = mybir.dt.float16
    BF16 = mybir.dt.bfloat16

    LS = n_steps
    NBLK = -(-LS // BLK)  # ceil

    # phase A tiles: (c, t_lo, t_wid), t-range-outer. t-tile 0 is split in
    # half so steps can start after only 32 half-width head tiles.
    tranges = [(0, 256), (256, 256), (512, 512), (1024, 512), (1536, 512)]
    tranges = [(lo, w) for lo, w in tranges if lo < 4 * LS]
    tilesA = [(c, lo, w) for lo, w in tranges for c in range(32)]
    n_tiles = len(tilesA)
    HEAD = min(32, n_tiles)  # (c, 0, 256) tiles computed before step 0

    # schedule interleaved (idx, k) matmul stream across steps, spreading
    # each t-range's matmuls evenly up to the step that first consumes it
    # (minus a pipeline margin), so the per-step overhead stays minimal.
    stepA_mms = [[] for _ in range(LS)]
    done_l = 0
    for lo, w in tranges:
        idxs = [i for i, (c, tlo, tw) in enumerate(tilesA)
                if tlo == lo and i >= HEAD]
        if not idxs:
            continue
        mms = [(idx, k) for idx in idxs for k in range(KT)]
        deadline = max(done_l + 1, min(LS, lo // 4 - 8))
        span = max(1, deadline - done_l)
        per = -(-len(mms) // span)
        p = 0
        for l in range(done_l, LS):
            if p >= len(mms):
                break
            stepA_mms[l].extend(mms[p : p + per])
            p += per
        assert p >= len(mms), "phase A does not fit in the step stream"
        done_l = min(deadline, done_l + -(-len(mms) // per))
    # step at which tile idx's last mm is emitted (head tiles: -1)
    tile_done_step = {idx: -1 for idx in range(HEAD)}
    for l in range(LS):
        for idx, k in stepA_mms[l]:
            if k == KT - 1:
                tile_done_step[idx] = l
    # evac for tile idx goes on ACT at step tile_done_step[idx]+1
    evac_sched = [[] for _ in range(LS)]
    head_evacs = []
    for idx in range(n_tiles):
        s = tile_done_step[idx] + 1
        if idx < HEAD:
            head_evacs.append(idx)
        else:
            evac_sched[min(s, LS - 1)].append(idx)
    # xgT store for tile idx goes on SP at evac step + 1
    store_sched = [[] for _ in range(LS)]
    head_stores = []
    for idx in range(n_tiles):
        if idx < HEAD:
            head_stores.append(idx)
        else:
            store_sched[min(tile_done_step[idx] + 2, LS - 1)].append(idx)

    nc = bacc.Bacc(None, target_bir_lowering=False)

    # ---- I/O ----
    xT = nc.dram_tensor("xT", [C, T], F16, kind="ExternalInput")
    wihT = nc.dram_tensor("wihT", [C, G4], F16, kind="ExternalInput")
    whhT = nc.dram_tensor("whhT", [C, G4], F16, kind="ExternalInput")
    bias2d = nc.dram_tensor("bias2d", [128, 32], F32, kind="ExternalInput")
    ident = nc.dram_tensor("ident", [128, 128], F16, kind="ExternalInput")
    yT = nc.dram_tensor("yT", [128, KT, T], F16, kind="ExternalOutput")
    xgT = nc.dram_tensor("xgT", [32, 128, T], F16)  # internal staging, 16MB

    with ExitStack() as es:
        sems = {}
        for nm in ("ld_sem ld2_sem gp_sem ev_sem wh_sem ms_sem id_sem "
                   "bs_sem lw0_sem lw1_sem lw2_sem lw3_sem xr0_sem xr1_sem xr2_sem xr3_sem "
                   "xt0_sem xt1_sem xt2_sem xt3_sem xs0_sem xs1_sem xs2_sem xs3_sem "
                   "pg_sem a_sem pt_sem c2s_sem igs_sem cs_sem hs_sem yw0_sem yw1_sem").split():
            sems[nm] = es.enter_context(nc.semaphore(nm))
        ld_sem, gp_sem, ev_sem = sems["ld_sem"], sems["gp_sem"], sems["ev_sem"]
        ld2_sem = sems["ld2_sem"]
        wh_sem, ms_sem, id_sem = sems["wh_sem"], sems["ms_sem"], sems["id_sem"]
        bs_sem = sems["bs_sem"]
        lw_sems = tuple(sems[f"lw{i}_sem"] for i in range(4))
        xr_sems = tuple(sems[f"xr{i}_sem"] for i in range(PF))
        xt_sems = tuple(sems[f"xt{i}_sem"] for i in range(4))
        xs_sems = tuple(sems[f"xs{i}_sem"] for i in range(4))
        pg_sem, a_sem, pt_sem = sems["pg_sem"], sems["a_sem"], sems["pt_sem"]
        c2s_sem, igs_sem = sems["c2s_sem"], sems["igs_sem"]
        cs_sem, hs_sem = sems["cs_sem"], sems["hs_sem"]
        yw_sems = (sems["yw0_sem"], sems["yw1_sem"])

        whh_sb = es.enter_context(nc.sbuf_tensor("whh_sb", [128, KT * G4], F16))
        wih_sb = es.enter_context(nc.sbuf_tensor("wih_sb", [128, KT * G4], F16))
        xT_sb = es.enter_context(nc.sbuf_tensor("xT_sb", [128, KT * T], F16))
        stage = es.enter_context(nc.sbuf_tensor("stage", [128, 4 * 512], F16))
        bias_sb = es.enter_context(nc.sbuf_tensor("bias_sb", [128, 32], F32))
        xg_sb = es.enter_context(nc.sbuf_tensor("xg_sb", [128, PF, 32, 4], F16))
        # h/y accumulator: [128, buf, k-tile, step-in-block*4]; fp16
        yacc = es.enter_context(nc.sbuf_tensor("yacc", [128, 2, KT, BLK * 4], F16))
        g_sb = es.enter_context(nc.sbuf_tensor("g_sb", [128, 128], F16))
        c_sb = es.enter_context(nc.sbuf_tensor("c_sb", [128, 32], F16))
        c2_sb = es.enter_context(nc.sbuf_tensor("c2_sb", [128, 32], F16))
        ig_sb = es.enter_context(nc.sbuf_tensor("ig_sb", [128, 32], F16))
        t_sb = es.enter_context(nc.sbuf_tensor("t_sb", [128, 2 * 32], F16))
        id_sb = es.enter_context(nc.sbuf_tensor("id_sb", [128, 128], F16))
        gps = es.enter_context(nc.psum_tensor("gps", [128, 2 * 512], F32))
        # 6 full 2KB psum banks: [if, g~, o] x 2 parities
        Gps = es.enter_context(nc.psum_tensor("Gps", [128, 6 * 512], F32))
        blk = es.enter_context(nc.Block())
        SIG = mybir.ActivationFunctionType.Sigmoid
        TANH = mybir.ActivationFunctionType.Tanh
        IDENT_F = mybir.ActivationFunctionType.Identity

        def h_rhs(l, k):
            """moving hT tile for step l's matmuls = h written at step l-1"""
            lp = l - 1
            b = (lp // BLK) % 2
            tb = lp % BLK  # python modulo: l=0 -> b=1, tb=BLK-1 (memset slot)
            return yacc[:, b, k, tb * 4 : (tb + 1) * 4]

        def emit_A_mm(t, idx, k):
            c, lo, w = tilesA[idx]
            bank = idx % 2  # psum bank
            if k == 0 and idx >= 2:
                t.wait_ge(ev_sem, idx - 1)  # psum bank free (evac of idx-2)
            mm = t.matmul(
                gps[:, bank * 512 : bank * 512 + w],
                wih_sb[:, k * G4 + c * 128 : k * G4 + (c + 1) * 128],
                xT_sb[:, k * T + lo : k * T + lo + w],
                start=(k == 0),
                stop=(k == KT - 1),
            )
            if k == KT - 1:
                mm.then_inc(gp_sem, 1)

        def emit_A_evac(s, idx):
            c, lo, w = tilesA[idx]
            pbank = idx % 2
            sslot = idx % 4
            s.wait_ge(gp_sem, idx + 1)
            if idx >= 4:
                # stage slot free: store of tile idx-4 completed
                wait_store_done(s, idx - 4)
            s.activation(
                stage[:, sslot * 512 : sslot * 512 + w],
                gps[:, pbank * 512 : pbank * 512 + w],
                IDENT_F,
                bias=bias_sb[:, c : c + 1],
            ).then_inc(ev_sem, 1)

        def emit_A_store(s, idx):
            c, lo, w = tilesA[idx]
            sslot = idx % 4
            s.wait_ge(ev_sem, idx + 1)
            sem = xt_sems[idx % 4] if idx < 32 else xs_sems[idx % 4]
            s.dma_start(
                xgT[c, :, lo : lo + w],
                stage[:, sslot * 512 : sslot * 512 + w],
            ).then_inc(sem, 16)

        def wait_store_done(e, j):
            if j < 32:
                e.wait_ge(xt_sems[j % 4], 16 * (j // 4 + 1))
            else:
                e.wait_ge(xs_sems[j % 4], 16 * ((j - 32) // 4 + 1))

        if_chunks = list(range(0, 16))
        g_chunks = list(range(16, 24))
        o_chunks = list(range(24, 32))

        @blk.gpsimd
        def _(g):
            for k in range(4):
                g.dma_start(
                    xT_sb[:, k * T : (k + 1) * T], xT[k * 128 : (k + 1) * 128, :]
                ).then_inc(ld_sem, 16)
            for k in range(KT):
                g.dma_start(
                    whh_sb[:, k * G4 : (k + 1) * G4], whhT[k * 128 : (k + 1) * 128, :]
                ).then_inc(wh_sem, 16)
            for l in range(LS):
                for idx in store_sched[l]:
                    emit_A_store(g, idx)

        @blk.sync
        def _(s):
            for q in range(4):  # gate quarters: chunks q*8..q*8+7
                ks = range(4) if q == 0 else range(KT)  # q0 k4..7 load on ACT
                for k in ks:
                    s.dma_start(
                        wih_sb[:, k * G4 + q * 1024 : k * G4 + (q + 1) * 1024],
                        wihT[k * 128 : (k + 1) * 128, q * 1024 : (q + 1) * 1024],
                    ).then_inc(lw_sems[q], 16)
                if q == 0:
                    for k in (4, 5):
                        s.dma_start(
                            xT_sb[:, k * T : (k + 1) * T],
                            xT[k * 128 : (k + 1) * 128, :],
                        ).then_inc(ld2_sem, 16)
                    s.dma_start(bias_sb[:, :], bias2d[:, :]).then_inc(bs_sem, 16)
                    s.dma_start(id_sb[:, :], ident[:, :]).then_inc(id_sem, 16)
            for l in range(LS):
                if l == 0:
                    for r in range(4):
                        s.wait_ge(xt_sems[r], 128)
                elif l in (64, 128, 256, 384):
                    m = {64: 1, 128: 2, 256: 3, 384: 4}[l]
                    for r in range(4):
                        s.wait_ge(xs_sems[r], 128 * m)
                if l >= PF:
                    s.wait_ge(pg_sem, 32 * (l - PF + 1))
                s.dma_start(
                    xg_sb[:, l % PF, :, :],
                    xgT[:, :, 4 * l : 4 * l + 4].transpose([1, 0, 2]),
                ).then_inc(xr_sems[l % PF], 16)
                b = (l - PF) // BLK if l - PF >= 0 else -1
                if b >= 0 and (l - PF + 1) % BLK == 0:
                    lo = b * BLK
                    hi = min(LS, lo + BLK)
                    s.wait_ge(hs_sem, hi)
                    s.dma_start(
                        yT[:, :, lo * 4 : hi * 4],
                        yacc[:, b % 2, :, 0 : (hi - lo) * 4],
                    ).then_inc(yw_sems[b % 2], 16)
            for b in range((LS - PF) // BLK if LS >= PF else 0, NBLK):
                lo = b * BLK
                hi = min(LS, lo + BLK)
                s.wait_ge(hs_sem, hi)
                s.dma_start(
                    yT[:, :, lo * 4 : hi * 4],
                    yacc[:, b % 2, :, 0 : (hi - lo) * 4],
                ).then_inc(yw_sems[b % 2], 16)
            s.wait_ge(yw_sems[0], 16 * ((NBLK + 1) // 2))
            if NBLK >= 2:
                s.wait_ge(yw_sems[1], 16 * (NBLK // 2))

        @blk.vector
        def _(v):
            v.memset(c_sb[:, :], 0.0).then_inc(ms_sem, 1)
            v.memset(yacc[:, 1, :, (BLK - 1) * 4 : BLK * 4], 0.0).then_inc(ms_sem, 1)
            for l in range(LS):
                # c2 = f * c
                v.wait_ge(a_sem, 4 * l + 1)
                if l >= 1:
                    v.wait_ge(cs_sem, l)
                v.tensor_mul(c2_sb[:, :], g_sb[:, 32:64], c_sb[:, :]).then_inc(
                    c2s_sem, 1
                )
                # ig = i * g~
                v.wait_ge(a_sem, 4 * l + 2)
                v.tensor_mul(ig_sb[:, :], g_sb[:, 0:32], g_sb[:, 96:128]).then_inc(
                    igs_sem, 1
                )
                # c = c2 + ig
                v.wait_ge(c2s_sem, l + 1)
                v.wait_ge(igs_sem, l + 1)
                v.tensor_add(c_sb[:, :], c2_sb[:, :], ig_sb[:, :]).then_inc(
                    cs_sem, 1
                )
                # h = o * tanh(c) -> fp16 hT layout, straight into yacc
                v.wait_ge(a_sem, 4 * (l + 1))
                b, tb = (l // BLK) % 2, l % BLK
                if tb == 0 and l // BLK >= 2:
                    bb = l // BLK
                    v.wait_ge(yw_sems[bb % 2], 16 * (bb // 2))
                v.tensor_mul(
                    yacc[:, b, :, tb * 4 : (tb + 1) * 4],
                    g_sb[:, 64:96],
                    t_sb[:, (l % 2) * 32 : (l % 2 + 1) * 32],
                ).then_inc(hs_sem, 1)

        @blk.tensor
        def _(t):
            t.wait_ge(ld_sem, 16 * 4)
            t.wait_ge(ld2_sem, 16 * 4)
            t.wait_ge(ms_sem, 2)
            for idx in range(HEAD):
                c = tilesA[idx][0]
                if idx == 0 or c // 8 != tilesA[idx - 1][0] // 8:
                    t.wait_ge(lw_sems[c // 8], 16 * KT)
                for k in range(KT):
                    emit_A_mm(t, idx, k)
            for q in range(4):
                t.wait_ge(lw_sems[q], 16 * KT)
            t.wait_ge(wh_sem, 16 * KT)
            t.wait_ge(id_sem, 16)
            for l in range(LS):
                t.wait_ge(xr_sems[l % PF], 16 * (l // PF + 1))
                if l >= 2:
                    t.wait_ge(a_sem, 4 * (l - 2) + 3)
                # big folds: seed each gate bank's psum with xgT (one open
                # accumulation group per 2KB bank), before h(l-1) is ready
                bA = (l % 2) * 512
                bB = (2 + l % 2) * 512
                bC = (4 + l % 2) * 512
                t.matmul(Gps[:, bA : bA + 64], id_sb[:, :],
                         xg_sb[:, l % PF, 0:16, :], start=True, stop=False)
                t.matmul(Gps[:, bB : bB + 32], id_sb[:, :],
                         xg_sb[:, l % PF, 16:24, :], start=True, stop=False)
                t.matmul(Gps[:, bC : bC + 32], id_sb[:, :],
                         xg_sb[:, l % PF, 24:32, :], start=True, stop=False)
                if l >= 1:
                    t.wait_ge(hs_sem, l)
                for c in if_chunks + g_chunks + o_chunks:
                    if c < 16:
                        bank = bA
                    elif c < 24:
                        bank = bB
                    else:
                        bank = bC
                    out = Gps[:, bank + _pcol(c) * 4 : bank + _pcol(c) * 4 + 4]
                    last_in_bank = c in (15, 23, 31)
                    for k in range(KT):
                        mm = t.matmul(
                            out,
                            whh_sb[:, k * G4 + c * 128 : k * G4 + (c + 1) * 128],
                            h_rhs(l, k),
                            start=False,
                            stop=(k == KT - 1 and last_in_bank),
                        )
                        if k == KT - 1:
                            mm.then_inc(pg_sem, 1)
                for idx, k in stepA_mms[l]:
                    emit_A_mm(t, idx, k)

        @blk.scalar
        def _(s):
            for k in range(4, KT):
                s.dma_start(
                    wih_sb[:, k * G4 : k * G4 + 1024],
                    wihT[k * 128 : (k + 1) * 128, 0:1024],
                ).then_inc(lw_sems[0], 16)
            for k in range(6, KT):
                s.dma_start(
                    xT_sb[:, k * T : (k + 1) * T], xT[k * 128 : (k + 1) * 128, :]
                ).then_inc(ld2_sem, 16)
            s.wait_ge(bs_sem, 16)
            for idx in head_evacs:
                emit_A_evac(s, idx)
                emit_A_store(s, idx)
            for l in range(LS):
                pb = (l % 2) * 512
                gb = (2 + l % 2) * 512
                ob = (4 + l % 2) * 512
                s.wait_ge(pg_sem, 32 * l + 16)
                if l >= 1:
                    s.wait_ge(hs_sem, l)
                s.activation(g_sb[:, 0:64], Gps[:, pb : pb + 64], SIG).then_inc(
                    a_sem, 1
                )
                s.wait_ge(pg_sem, 32 * l + 24)
                s.activation(
                    g_sb[:, 96:128], Gps[:, gb : gb + 32], TANH
                ).then_inc(a_sem, 1)
                s.wait_ge(pg_sem, 32 * (l + 1))
                s.activation(g_sb[:, 64:96], Gps[:, ob : ob + 32], SIG).then_inc(
                    a_sem, 1
                )
                s.wait_ge(cs_sem, l + 1)
                s.activation(
                    t_sb[:, (l % 2) * 32 : (l % 2 + 1) * 32], c_sb[:, :], TANH
                ).then_inc(a_sem, 1)
                for idx in evac_sched[l]:
                    emit_A_evac(s, idx)

    nc.finalize()
    return nc


def _get_nc(n_steps):
    if n_steps not in _cached:
        _cached[n_steps] = _build(n_steps)
    return _cached[n_steps]


def _host_inputs(x, W_ih, W_hh, b_ih, b_hh):
    import ml_dtypes

    BF = ml_dtypes.bfloat16
    x = np.asarray(x, np.float32)
    W_ih = np.asarray(W_ih, np.float32)
    W_hh = np.asarray(W_hh, np.float32)
    bias = np.asarray(b_ih, np.float32) + np.asarray(b_hh, np.float32)

    wihT = np.ascontiguousarray(W_ih.T).astype(np.float16)
    whhT16 = W_hh.T.astype(np.float16)
    bias2d = np.ascontiguousarray(bias.reshape(32, 128).T)
    ident = np.eye(128, dtype=np.float16)

    in_maps = []
    for c in range(NCORES):
        xT_c = np.ascontiguousarray(x[c].T).astype(np.float16)
        in_maps.append(
            {"xT": xT_c, "wihT": wihT, "whhT": whhT16, "bias2d": bias2d,
             "ident": ident}
        )
    return in_maps


def kernel(x, W_ih, W_hh, b_ih, b_hh):
    from concourse.bass_utils import run_bass_kernel_spmd

    nc = _get_nc(N_STEPS)
    in_maps = _host_inputs(x, W_ih, W_hh, b_ih, b_hh)

    res = run_bass_kernel_spmd(nc, in_maps, list(range(NCORES)))
    outs = []
    for c in range(NCORES):
        yT_p = np.asarray(res.results[c]["yT"], dtype=np.float32)  # [128, 8, T]
        outs.append(yT_p.transpose(2, 1, 0).reshape(T, H))
    return np.stack(outs, axis=0).astype(np.float32)

